# revision 1
# baseline (speedup 1.0000x reference)
"""ColorMLP Trainium2 kernel.

Reference computation (per pixel, 8 input channels):
    h1 = relu(x @ w0 + b0)         # 8 -> 16
    h2 = relu(h1 @ w1 + b1)        # 16 -> 16
    y  = sigmoid(h2 @ w2 + b2)     # 16 -> 3
    out = mask * ((1-res)*rgb + res*y)   rgb = x[..,:3], res = x[..,3]

Strategy (pure data parallel over 8 cores, 1,048,576 px each):
  - SWDGE cast-DMA loads x as bf16 pixel-major [128, 2048] per 32768-px batch.
  - PE transposes [128,128] bf16 chunks -> feature-major t [128=16grp x 8f, 512].
  - L0: 64x128 row-tiled block-diag(w0 x8) matmuls (2 concurrent tiles).
  - L1: full-array block-diag(w1 x8) matmul.
  - L2 fused with the output transpose: lhsT = relu(h2) chunk [128,128],
    rhs = block-diag(w2 x8) [128,24] -> pre-sigmoid z lands PIXEL-MAJOR in PSUM.
  - ACT sigmoid -> y_sb pixel-major bf16; DVE blend; SWDGE cast-DMA store f32.
"""

import os
import sys

import numpy as np

sys.path.insert(0, "/opt/trn_rl_repo")

import ml_dtypes

import concourse.bacc as bacc
import concourse.bass as bass
import concourse.mybir as mybir
import concourse.tile as tile
from concourse.bass_utils import run_bass_kernel_spmd

F32 = mybir.dt.float32
BF16 = mybir.dt.bfloat16
U8 = mybir.dt.uint8

N_CORES = 8
B, H, W = 8, 1024, 1024
IN_DIM, HID, OUT_DIM = 8, 16, 3
NPX = B * H * W                  # 8388608
NPC = NPX // N_CORES             # 1048576 per core

BATCH_PX = 32768                 # pixels per batch (4 supertiles)
PPP = BATCH_PX // 128            # 256 px per partition per batch
N_ST = 4                         # supertiles per batch (each 8192 px)
# per supertile: 4 transpose chunks of 128 cols; 16 pixel-groups per chunk

# Engine assignment: relu1 always ScalarE, relu2 always VectorE. Keeping one
# writer engine per tile tag means same-engine WAW/WAR edges stay no-sync and
# every datapath instruction fits the single hw sync-wait slot.


def _bd(w, reps):
    """Block-diagonal of `w` repeated `reps` times: [reps*K, reps*M]."""
    k, m = w.shape
    out = np.zeros((reps * k, reps * m), np.float32)
    for g in range(reps):
        out[g * k:(g + 1) * k, g * m:(g + 1) * m] = w
    return out


def _prep_weights(w0, b0, w1, b1, w2, b2):
    """Host-side constant prep. Returns dict of named numpy arrays."""
    bf = ml_dtypes.bfloat16
    # W0T [128,128]: rows 0-63 & 64-127 are each blockdiag8(w0) [64,128]
    bd0 = _bd(w0, 8)  # [64, 128]
    w0t = np.concatenate([bd0, bd0], axis=0)  # [128, 128]
    # W1BD [128,128] = blockdiag8(w1)
    w1bd = _bd(w1, 8)
    # G2 [128, 24] = blockdiag8(w2)
    g2 = _bd(w2, 8)  # [128, 24]
    ident = np.eye(128, dtype=np.float32)
    # per-partition biases for h rows (g,j) -> b[j]
    b0col = np.tile(b0, 8).astype(np.float32).reshape(128, 1)
    b1col = np.tile(b1, 8).astype(np.float32).reshape(128, 1)
    # b2 pattern along free dim of z: (c*16 + s*8 + g)*3 + ch for one batch-row
    # of 192 cols per supertile -> b2 tiled 64x
    b2row = np.tile(b2, 64).astype(np.float32).reshape(1, 192)
    return {
        "W0T": w0t.astype(bf),
        "W1BD": w1bd.astype(bf),
        "G2": g2.astype(bf),
        "IDENT": ident.astype(bf),
        "B0COL": b0col,
        "B1COL": b1col,
        "B2ROW": b2row,
        "b01_nonzero": bool(np.any(b0 != 0.0) or np.any(b1 != 0.0)),
        "b2_nonzero": bool(np.any(b2 != 0.0)),
    }


def build_program(npc, b01_nonzero, b2_nonzero):
    """Build the SPMD Bass program for one core processing `npc` pixels."""
    nc = bacc.Bacc("TRN2", target_bir_lowering=False, debug=False,
                   num_devices=N_CORES)
    n_batch = npc // BATCH_PX

    x_d = nc.dram_tensor("x", [npc, IN_DIM], F32, kind="ExternalInput")
    m_d = nc.dram_tensor("mask", [npc], U8, kind="ExternalInput")
    w0t_d = nc.dram_tensor("W0T", [128, 128], BF16, kind="ExternalInput")
    w1bd_d = nc.dram_tensor("W1BD", [128, 128], BF16, kind="ExternalInput")
    g2_d = nc.dram_tensor("G2", [128, 24], BF16, kind="ExternalInput")
    id_d = nc.dram_tensor("IDENT", [128, 128], BF16, kind="ExternalInput")
    b0_d = nc.dram_tensor("B0COL", [128, 1], F32, kind="ExternalInput")
    b1_d = nc.dram_tensor("B1COL", [128, 1], F32, kind="ExternalInput")
    b2_d = nc.dram_tensor("B2ROW", [1, 192], F32, kind="ExternalInput")
    out_d = nc.dram_tensor("out", [npc, OUT_DIM], F32, kind="ExternalOutput")

    # DRAM views
    # batch b, partition p covers pixels b*BATCH_PX + p*PPP + [0, PPP)
    x_v = x_d[:].rearrange("(b p n) f -> b p (n f)", b=n_batch, p=128)
    m_v = m_d[:].rearrange("(b p n) -> b p n", b=n_batch, p=128)
    o_v = out_d[:].rearrange("(b p n) c -> b p (n c)", b=n_batch, p=128)

    with tile.TileContext(nc) as tc:
        with (
            tc.tile_pool(name="consts", bufs=1) as cpool,
            tc.tile_pool(name="xin", bufs=3) as xpool,
            tc.tile_pool(name="msk", bufs=3) as mpool,
            tc.tile_pool(name="tsb", bufs=2) as tpool,
            tc.tile_pool(name="hsb", bufs=2) as hpool,
            tc.tile_pool(name="h2sb", bufs=2) as h2pool,
            tc.tile_pool(name="ysb", bufs=2) as ypool,
            tc.tile_pool(name="blend", bufs=2) as bpool,
            tc.tile_pool(name="osb", bufs=3) as opool,
            tc.tile_pool(name="tps", bufs=1, space="PSUM") as tps_pool,
            tc.tile_pool(name="hps", bufs=1, space="PSUM") as hps_pool,
            tc.tile_pool(name="h2ps", bufs=1, space="PSUM") as h2ps_pool,
            tc.tile_pool(name="zps", bufs=1, space="PSUM") as zps_pool,
        ):
            # ---- load constants once ----
            w0t = cpool.tile([128, 128], BF16, tag="w0t")
            w1bd = cpool.tile([128, 128], BF16, tag="w1bd")
            g2 = cpool.tile([128, 24], BF16, tag="g2")
            ident = cpool.tile([128, 128], BF16, tag="ident")
            nc.sync.dma_start(w0t[:], w0t_d[:])
            nc.sync.dma_start(w1bd[:], w1bd_d[:])
            nc.sync.dma_start(g2[:], g2_d[:])
            nc.sync.dma_start(ident[:], id_d[:])
            if b01_nonzero:
                b0c = cpool.tile([128, 1], F32, tag="b0c")
                b1c = cpool.tile([128, 1], F32, tag="b1c")
                nc.sync.dma_start(b0c[:], b0_d[:])
                nc.sync.dma_start(b1c[:], b1_d[:])
            if b2_nonzero:
                b2r = cpool.tile([1, 192], F32, tag="b2r")
                nc.sync.dma_start(b2r[:], b2_d[:])

            def relu_evac(dst, src, bias_tile, on_act):
                # zero biases use the pre-registered const-0 AP (float arg),
                # which carries no runtime DMA dependency
                if on_act:
                    bias = bias_tile[:] if bias_tile is not None else 0.0
                    nc.scalar.activation(
                        dst, src, mybir.ActivationFunctionType.Relu, bias=bias)
                else:
                    s1 = bias_tile[:] if bias_tile is not None else 0.0
                    nc.vector.tensor_scalar(
                        out=dst, in0=src, scalar1=s1, scalar2=0.0,
                        op0=mybir.AluOpType.add, op1=mybir.AluOpType.max)

            for b in range(n_batch):
                # ---- loads (SWDGE cast f32->bf16, u8->bf16) ----
                x_bf = xpool.tile([128, PPP * IN_DIM], BF16, tag="x")   # [128,2048]
                nc.gpsimd.dma_start(x_bf[:], x_v[b])
                mask_bf = mpool.tile([128, PPP], BF16, tag="m")
                nc.gpsimd.dma_start(mask_bf[:], m_v[b])

                y_sb = ypool.tile([128, PPP * 3], BF16, tag="y")        # [128,768]
                z_ps = zps_pool.tile([128, 1024], F32, tag="z")
                for st in range(N_ST):
                    # ---- transposes: 4 chunks of [128,128] ----
                    t_ps = tps_pool.tile([128, 512], BF16, tag="tp")
                    t_sb = tpool.tile([128, 512], BF16, tag="t")
                    for c in range(4):
                        nc.tensor.transpose(
                            t_ps[:, c * 128:(c + 1) * 128],
                            x_bf[:, st * 512 + c * 128: st * 512 + (c + 1) * 128],
                            ident[:],
                        )
                    nc.vector.tensor_copy(t_sb[:], t_ps[:])

                    for s in range(2):
                        # ---- L0 (64x128 row-tiled, 2 concurrent tiles) ----
                        h_ps = hps_pool.tile([128, 512], F32, tag=f"h{s}")
                        nc.tensor.matmul(
                            h_ps[:],
                            lhsT=w0t[s * 64:(s + 1) * 64, :],
                            rhs=t_sb[s * 64:(s + 1) * 64, :],
                        )
                        h_sb = hpool.tile([128, 512], BF16, tag=f"hs{s}")
                        relu_evac(h_sb[:], h_ps[:], b0c if b01_nonzero else None, True)

                        # ---- L1 (full array) ----
                        h2_ps = h2ps_pool.tile([128, 512], F32, tag=f"h2{s}")
                        nc.tensor.matmul(h2_ps[:], lhsT=w1bd[:], rhs=h_sb[:])
                        h2_sb = h2pool.tile([128, 512], BF16, tag=f"h2s{s}")
                        relu_evac(h2_sb[:], h2_ps[:], b1c if b01_nonzero else None, False)

                        # ---- L2 fused with out-transpose ----
                        # z[p, slot]: pixel p*PPP + st*64 + c*16 + s*8 + g,
                        # slot = (st*64 + c*16 + s*8 + g)*3 + ch mapped below
                        for c in range(4):
                            col = st * 256 + c * 48 + s * 24
                            nc.tensor.matmul(
                                z_ps[:, col:col + 24],
                                lhsT=h2_sb[:, c * 128:(c + 1) * 128],
                                rhs=g2[:],
                            )

                # optional +b2 (skipped when b2 == 0)
                z_view = z_ps[:].rearrange("p (st k) -> p st k", st=4)[:, :, :192]
                if b2_nonzero:
                    nc.vector.tensor_tensor(
                        out=z_view,
                        in0=z_view,
                        in1=b2r[:].partition_broadcast(128).rearrange(
                            "p k -> p 1 k").broadcast_to([128, 4, 192]),
                        op=mybir.AluOpType.add,
                    )

                # ---- sigmoid (one op per batch, strided PSUM view) ----
                # The 1-col touch absorbs the WAR wait on the recycled y_sb
                # slot (a DVE reader) so the sigmoid itself only waits on the
                # PE semaphore; the WAW on col 0 orders touch before sigmoid.
                nc.scalar.activation(
                    y_sb[:, 0:1], y_sb[:, 0:1],
                    mybir.ActivationFunctionType.Relu)
                nc.scalar.activation(
                    y_sb[:].rearrange("p (st k) -> p st k", st=4),
                    z_view,
                    mybir.ActivationFunctionType.Sigmoid,
                )

                # ---- blend: out = a*rgb + b*y,  a = mask*(1-res), b = mask*res
                x3 = x_bf[:].rearrange("p (n f) -> p n f", f=IN_DIM)
                rgb = x3[:, :, 0:3]
                res = x3[:, :, 3]
                bco = bpool.tile([128, PPP], BF16, tag="bc")
                aco = bpool.tile([128, PPP], BF16, tag="ac")
                nres = bpool.tile([128, PPP], BF16, tag="nr")
                # (1-res) via tensor_scalar first: TS tolerates 2 sem waits, so
                # the DVE observes the x-DMA sem here; the TTs below (1-wait
                # ISA slot) then each need at most one new sem.
                nc.vector.tensor_scalar(
                    out=nres[:], in0=res, scalar1=-1.0, scalar2=1.0,
                    op0=mybir.AluOpType.mult, op1=mybir.AluOpType.add)
                nc.vector.tensor_tensor(out=bco[:], in0=res, in1=mask_bf[:],
                                        op=mybir.AluOpType.mult)
                nc.vector.tensor_tensor(out=aco[:], in0=nres[:], in1=mask_bf[:],
                                        op=mybir.AluOpType.mult)
                u_sb = bpool.tile([128, PPP * 3], BF16, tag="u")
                v_sb = bpool.tile([128, PPP * 3], BF16, tag="v")
                o_sb = opool.tile([128, PPP * 3], BF16, tag="o")
                u3 = u_sb[:].rearrange("p (n c) -> p n c", c=3)
                v3 = v_sb[:].rearrange("p (n c) -> p n c", c=3)
                y3 = y_sb[:].rearrange("p (n c) -> p n c", c=3)
                nc.vector.tensor_tensor(
                    out=u3, in0=rgb,
                    in1=aco[:].unsqueeze(2).broadcast_to([128, PPP, 3]),
                    op=mybir.AluOpType.mult)
                nc.vector.tensor_tensor(
                    out=v3, in0=y3,
                    in1=bco[:].unsqueeze(2).broadcast_to([128, PPP, 3]),
                    op=mybir.AluOpType.mult)
                nc.vector.tensor_tensor(out=o_sb[:], in0=v_sb[:], in1=u_sb[:],
                                        op=mybir.AluOpType.add)

                # ---- store (SWDGE cast bf16->f32) ----
                nc.gpsimd.dma_start(o_v[b], o_sb[:])

    nc.finalize()
    return nc


_PROGRAM_CACHE = {}


def _get_program(npc, b01_nonzero, b2_nonzero):
    key = (npc, b01_nonzero, b2_nonzero)
    if key not in _PROGRAM_CACHE:
        _PROGRAM_CACHE[key] = build_program(npc, b01_nonzero, b2_nonzero)
    return _PROGRAM_CACHE[key]


def kernel(x, mask, w0, b0, w1, b1, w2, b2):
    x = np.asarray(x, np.float32)
    mask_u8 = np.asarray(mask).astype(np.uint8)
    consts = _prep_weights(
        np.asarray(w0, np.float32), np.asarray(b0, np.float32),
        np.asarray(w1, np.float32), np.asarray(b1, np.float32),
        np.asarray(w2, np.float32), np.asarray(b2, np.float32))

    x_flat = np.ascontiguousarray(x.reshape(NPX, IN_DIM))
    m_flat = np.ascontiguousarray(mask_u8.reshape(NPX))

    nc = _get_program(NPC, consts["b01_nonzero"], consts["b2_nonzero"])
    const_map = {k: np.asarray(v) for k, v in consts.items()
                 if k not in ("b2_nonzero", "b01_nonzero")}
    in_maps = []
    for k in range(N_CORES):
        lo, hi = k * NPC, (k + 1) * NPC
        in_maps.append({
            "x": x_flat[lo:hi],
            "mask": m_flat[lo:hi],
            **const_map,
        })
    res = run_bass_kernel_spmd(nc, in_maps, core_ids=list(range(N_CORES)))
    out = np.concatenate([res.results[k]["out"] for k in range(N_CORES)], axis=0)
    return out.reshape(B, H, W, OUT_DIM)


def _build_in_maps(x, mask, w0, b0, w1, b1, w2, b2):
    x = np.asarray(x, np.float32)
    mask_u8 = np.asarray(mask).astype(np.uint8)
    consts = _prep_weights(
        np.asarray(w0, np.float32), np.asarray(b0, np.float32),
        np.asarray(w1, np.float32), np.asarray(b1, np.float32),
        np.asarray(w2, np.float32), np.asarray(b2, np.float32))
    x_flat = np.ascontiguousarray(x.reshape(NPX, IN_DIM))
    m_flat = np.ascontiguousarray(mask_u8.reshape(NPX))
    nc = _get_program(NPC, consts["b01_nonzero"], consts["b2_nonzero"])
    const_map = {k: np.asarray(v) for k, v in consts.items()
                 if k not in ("b2_nonzero", "b01_nonzero")}
    in_maps = []
    for k in range(N_CORES):
        lo, hi = k * NPC, (k + 1) * NPC
        in_maps.append({"x": x_flat[lo:hi], "mask": m_flat[lo:hi], **const_map})
    return nc, in_maps


def run_traced(**inputs):
    """Run with NTFF tracing; returns the BassKernelResults (exec_time_ns)."""
    nc, in_maps = _build_in_maps(**inputs)
    return run_bass_kernel_spmd(
        nc, in_maps, core_ids=list(range(N_CORES)), trace=True,
        stitch_traces=False)


if __name__ == "__main__":
    # quick smoke test with random data
    rng = np.random.default_rng(0)
    x = rng.random((B, H, W, IN_DIM), np.float32)
    mask = rng.integers(0, 2, (B, H, W)).astype(bool)
    w0 = rng.standard_normal((IN_DIM, HID)).astype(np.float32) * 0.5
    b0 = np.zeros(HID, np.float32)
    w1 = rng.standard_normal((HID, HID)).astype(np.float32) * 0.3
    b1 = np.zeros(HID, np.float32)
    w2 = rng.standard_normal((HID, OUT_DIM)).astype(np.float32) * 0.3
    b2 = np.zeros(OUT_DIM, np.float32)
    out = kernel(x=x, mask=mask, w0=w0, b0=b0, w1=w1, b1=b1, w2=w2, b2=b2)
    print("out", out.shape, out.dtype, out[0, 0, :2])



# revision 21
# speedup vs baseline: 18.0138x; 18.0138x over previous
"""ColorMLP Trainium2 kernel (v2 — engine-rebalanced).

Reference computation (per pixel, 8 input channels):
    h1 = relu(x @ w0 + b0)         # 8 -> 16
    h2 = relu(h1 @ w1 + b1)        # 16 -> 16
    y  = sigmoid(h2 @ w2 + b2)     # 16 -> 3
    out = mask * ((1-res)*rgb + res*y)   rgb = x[..,:3], res = x[..,3]

Strategy (pure data parallel over 8 cores, 1,048,576 px each):
  Pixel->partition map is partition-contiguous: partition p owns pixels
  [p*8192, (p+1)*8192) of the core's slice; batch b covers within-partition
  offsets [b*256, (b+1)*256).

  - mask: ONE SWDGE cast-DMA u8->bf16 for the whole core [128, 8192].
  - x: per-batch SWDGE cast-DMA f32->bf16 pixel-major [128, 2048].
  - PE transposes x chunks -> t_ps (bf16 PSUM); single DVE evac (2x mode).
  - L0/L1 matmuls write BF16 PSUM so evacs run in DVE 2x_1p / cheap ACT.
    relu1 evac on DVE (tensor_scalar max, 2x); relu2 evac on ACT (Relu).
  - L2 fused with output transpose (stationary = h2 chunk): z lands
    pixel-major f32 in PSUM; ACT sigmoid -> y_sb bf16.
  - blend split: nres+v+o on DVE, aco/bco/u on GpSimd (SBUF-only TTs).
  - store: o_sb f32 via HWDGE (nc.sync) — no SWDGE descriptor cost.
"""

import os
import sys

import numpy as np

sys.path.insert(0, "/opt/trn_rl_repo")

import ml_dtypes

import concourse.bacc as bacc
import concourse.bass as bass
import concourse.mybir as mybir
import concourse.tile as tile
from concourse.bass_utils import run_bass_kernel_spmd

F32 = mybir.dt.float32
BF16 = mybir.dt.bfloat16
U8 = mybir.dt.uint8

N_CORES = 8
B, H, W = 8, 1024, 1024
IN_DIM, HID, OUT_DIM = 8, 16, 3
NPX = B * H * W                  # 8388608
NPC = NPX // N_CORES             # 1048576 per core
PPPC = NPC // 128                # 8192 pixels per partition per core

BATCH_PX = 32768                 # pixels per batch
PPP = BATCH_PX // 128            # 256 px per partition per batch


def _bd(w, reps):
    """Block-diagonal of `w` repeated `reps` times: [reps*K, reps*M]."""
    k, m = w.shape
    out = np.zeros((reps * k, reps * m), np.float32)
    for g in range(reps):
        out[g * k:(g + 1) * k, g * m:(g + 1) * m] = w
    return out


def _prep_weights(w0, b0, w1, b1, w2, b2):
    """Host-side constant prep. Returns dict of named numpy arrays."""
    bf = ml_dtypes.bfloat16
    # W0T [128,128]: rows 0-63 & 64-127 are each blockdiag8(w0) [64,128]
    bd0 = _bd(w0, 8)  # [64, 128]
    w0t = np.concatenate([bd0, bd0], axis=0)  # [128, 128]
    w1bd = _bd(w1, 8)  # [128, 128]
    g2 = _bd(w2, 8)    # [128, 24]
    ident = np.eye(128, dtype=np.float32)
    b0col = np.tile(b0, 8).astype(np.float32).reshape(128, 1)
    b1col = np.tile(b1, 8).astype(np.float32).reshape(128, 1)
    # b2 pattern along the z free dim: slot = n*3 + ch, n in [0,256)
    b2row = np.tile(b2, 256).astype(np.float32).reshape(1, 768)
    return {
        "W0T": w0t.astype(bf),
        "W1BD": w1bd.astype(bf),
        "G2": g2.astype(bf),
        "IDENT": ident.astype(bf),
        "B0COL": b0col,
        "B1COL": b1col,
        "B2ROW": b2row,
        "b01_nonzero": bool(np.any(b0 != 0.0) or np.any(b1 != 0.0)),
        "b2_nonzero": bool(np.any(b2 != 0.0)),
    }


def build_program(npc, b01_nonzero, b2_nonzero, repeat=1):
    """Build the SPMD Bass program for one core processing `npc` pixels.

    `repeat` re-runs the whole pipeline R times (same input, same output)
    — bench-only knob to cancel host dispatch overhead out of wall-clock
    timing: device_time = (wall(R) - wall(1)) / (R - 1).
    """
    nc = bacc.Bacc("TRN2", target_bir_lowering=False, debug=False,
                   num_devices=N_CORES)
    n_batch = npc // BATCH_PX
    pppc = npc // 128

    x_d = nc.dram_tensor("x", [npc, IN_DIM], F32, kind="ExternalInput")
    m_d = nc.dram_tensor("mask", [npc], U8, kind="ExternalInput")
    w0t_d = nc.dram_tensor("W0T", [128, 128], BF16, kind="ExternalInput")
    w1bd_d = nc.dram_tensor("W1BD", [128, 128], BF16, kind="ExternalInput")
    g2_d = nc.dram_tensor("G2", [128, 24], BF16, kind="ExternalInput")
    id_d = nc.dram_tensor("IDENT", [128, 128], BF16, kind="ExternalInput")
    b0_d = nc.dram_tensor("B0COL", [128, 1], F32, kind="ExternalInput")
    b1_d = nc.dram_tensor("B1COL", [128, 1], F32, kind="ExternalInput")
    b2_d = nc.dram_tensor("B2ROW", [1, 768], F32, kind="ExternalInput")
    out_d = nc.dram_tensor("out", [npc, OUT_DIM], F32, kind="ExternalOutput")

    # DRAM views — partition-contiguous pixel map:
    # partition p owns pixels p*pppc + [0, pppc); batch b covers
    # within-partition offsets b*PPP + [0, PPP).
    x_v = x_d[:].rearrange("(p b n) f -> b p (n f)", p=128, b=n_batch)
    m_v = m_d[:].rearrange("(p n) -> p n", p=128)
    o_v = out_d[:].rearrange("(p b n) c -> b p (n c)", p=128, b=n_batch)

    with tile.TileContext(nc) as tc:
        with (
            tc.tile_pool(name="consts", bufs=1) as cpool,
            tc.tile_pool(name="xin", bufs=3) as xpool,
            tc.tile_pool(name="tsb", bufs=2) as tpool,
            tc.tile_pool(name="hsb", bufs=2) as hpool,
            tc.tile_pool(name="h2sb", bufs=2) as h2pool,
            tc.tile_pool(name="ysb", bufs=2) as ypool,
            tc.tile_pool(name="blend", bufs=2) as bpool,
            tc.tile_pool(name="osb", bufs=3) as opool,
            tc.tile_pool(name="tps", bufs=2, space="PSUM") as tps_pool,
            tc.tile_pool(name="hps", bufs=1, space="PSUM") as hps_pool,
            tc.tile_pool(name="h2ps", bufs=1, space="PSUM") as h2ps_pool,
            tc.tile_pool(name="zps", bufs=1, space="PSUM") as zps_pool,
        ):
            # ---- constants + whole-core mask (u8 -> bf16 cast DMA) ----
            w0t = cpool.tile([128, 128], BF16, tag="w0t")
            w1bd = cpool.tile([128, 128], BF16, tag="w1bd")
            g2 = cpool.tile([128, 24], BF16, tag="g2")
            ident = cpool.tile([128, 128], BF16, tag="ident")
            mask_sb = cpool.tile([128, pppc], BF16, tag="mask")
            nc.sync.dma_start(w0t[:], w0t_d[:])
            nc.sync.dma_start(w1bd[:], w1bd_d[:])
            nc.sync.dma_start(g2[:], g2_d[:])
            nc.sync.dma_start(ident[:], id_d[:])
            nc.gpsimd.dma_start(mask_sb[:], m_v)
            if b01_nonzero:
                b0c = cpool.tile([128, 1], F32, tag="b0c")
                b1c = cpool.tile([128, 1], F32, tag="b1c")
                nc.sync.dma_start(b0c[:], b0_d[:])
                nc.sync.dma_start(b1c[:], b1_d[:])
            if b2_nonzero:
                b2r = cpool.tile([1, 768], F32, tag="b2r")
                nc.sync.dma_start(b2r[:], b2_d[:])

            for b in [b for _ in range(repeat) for b in range(n_batch)]:
                # ---- load x (SWDGE cast f32->bf16), pixel-major ----
                x_bf = xpool.tile([128, PPP * IN_DIM], BF16, tag="x")  # [128,2048]
                nc.gpsimd.dma_start(x_bf[:], x_v[b])

                # ---- transposes: 16 chunks of [128,128], 2 waves -> t_sb ----
                t_sb = tpool.tile([128, 2048], BF16, tag="t")
                for w in range(2):
                    t_ps = tps_pool.tile([128, 1024], BF16, tag="tp")
                    for c in range(8):
                        nc.tensor.transpose(
                            t_ps[:, c * 128:(c + 1) * 128],
                            x_bf[:, (w * 8 + c) * 128:(w * 8 + c + 1) * 128],
                            ident[:],
                        )
                    nc.vector.tensor_copy(
                        t_sb[:, w * 1024:(w + 1) * 1024], t_ps[:])  # 2x

                # blend coefficients early — they only need x and mask, and
                # keeping them off the batch tail lets batches overlap.
                x3 = x_bf[:].rearrange("p (n f) -> p n f", f=IN_DIM)
                rgb = x3[:, :, 0:3]
                res = x3[:, :, 3]
                mk = mask_sb[:, b * PPP:(b + 1) * PPP]
                nres = bpool.tile([128, PPP], BF16, tag="nr")
                bco = bpool.tile([128, PPP], BF16, tag="bc")
                aco = bpool.tile([128, PPP], BF16, tag="ac")
                u_sb = bpool.tile([128, PPP * 3], BF16, tag="u")
                nc.vector.tensor_scalar(
                    out=nres[:], in0=res, scalar1=-1.0, scalar2=1.0,
                    op0=mybir.AluOpType.mult, op1=mybir.AluOpType.add)
                nc.gpsimd.tensor_tensor(out=bco[:], in0=res, in1=mk,
                                        op=mybir.AluOpType.mult)
                nc.gpsimd.tensor_tensor(out=aco[:], in0=nres[:], in1=mk,
                                        op=mybir.AluOpType.mult)
                u3 = u_sb[:].rearrange("p (n c) -> p n c", c=3)
                nc.gpsimd.tensor_tensor(
                    out=u3, in0=rgb,
                    in1=aco[:].unsqueeze(2).broadcast_to([128, PPP, 3]),
                    op=mybir.AluOpType.mult)

                y_sb = ypool.tile([128, PPP * 3], BF16, tag="y")       # [128,768]
                z_ps = zps_pool.tile([128, 768], F32, tag="z")

                def relu_evac(dst, src, bias_tile, on_act):
                    if on_act:
                        bias = bias_tile[:] if bias_tile is not None else 0.0
                        nc.scalar.activation(
                            dst, src, mybir.ActivationFunctionType.Relu,
                            bias=bias)
                    else:
                        s1 = bias_tile[:] if bias_tile is not None else 0.0
                        nc.vector.tensor_scalar(
                            out=dst, in0=src, scalar1=s1, scalar2=0.0,
                            op0=mybir.AluOpType.add, op1=mybir.AluOpType.max)

                # evac engine split across the 8 quarters (per batch):
                # 5 on ACT, 3 on DVE — roughly balances ACT (sigmoid) vs
                # DVE (t-evac + blend) load.
                r1_on_act = {0: True, 1: False, 2: True, 3: False}
                r2_on_act = {0: True, 1: True, 2: True, 3: False}

                for s in range(2):
                    b0t = b0c if b01_nonzero else None
                    b1t = b1c if b01_nonzero else None
                    h_sb = hpool.tile([128, 2048], BF16, tag=f"hs{s}")
                    for hh in range(2):
                        # ---- L0 (64-row tiles, f32 PSUM, N=512 x2) ----
                        h_ps = hps_pool.tile([128, 1024], F32, tag="h")
                        for q in range(2):
                            lo = hh * 1024 + q * 512
                            nc.tensor.matmul(
                                h_ps[:, q * 512:(q + 1) * 512],
                                lhsT=w0t[s * 64:(s + 1) * 64, :],
                                rhs=t_sb[s * 64:(s + 1) * 64, lo:lo + 512],
                            )
                        relu_evac(h_sb[:, hh * 1024:(hh + 1) * 1024], h_ps[:],
                                  b0t, r1_on_act[s * 2 + hh])

                    h2_sb = h2pool.tile([128, 2048], BF16, tag=f"h2s{s}")
                    for hh in range(2):
                        # ---- L1 (full array, f32 PSUM, N=512 x2) ----
                        h2_ps = h2ps_pool.tile([128, 1024], F32, tag="h2")
                        for q in range(2):
                            lo = hh * 1024 + q * 512
                            nc.tensor.matmul(
                                h2_ps[:, q * 512:(q + 1) * 512],
                                lhsT=w1bd[:],
                                rhs=h_sb[:, lo:lo + 512],
                            )
                        relu_evac(h2_sb[:, hh * 1024:(hh + 1) * 1024],
                                  h2_ps[:], b1t, r2_on_act[s * 2 + hh])

                    # ---- L2 fused with out-transpose ----
                    # chunk cc holds px n = cc*16 + s*8 + g (partition = j);
                    # z slot = n*3 + ch  ->  base col = cc*48 + s*24
                    for cc in range(16):
                        col = cc * 48 + s * 24
                        nc.tensor.matmul(
                            z_ps[:, col:col + 24],
                            lhsT=h2_sb[:, cc * 128:(cc + 1) * 128],
                            rhs=g2[:],
                        )

                if b2_nonzero:
                    nc.vector.tensor_tensor(
                        out=z_ps[:], in0=z_ps[:],
                        in1=b2r[:].partition_broadcast(128),
                        op=mybir.AluOpType.add)

                # ---- sigmoid + blend tail ----
                # 1-col touch absorbs the WAR wait on the recycled y_sb slot.
                nc.scalar.activation(
                    y_sb[:, 0:1], y_sb[:, 0:1],
                    mybir.ActivationFunctionType.Relu)
                nc.scalar.activation(
                    y_sb[:], z_ps[:],
                    mybir.ActivationFunctionType.Sigmoid,
                )
                v_sb = bpool.tile([128, PPP * 3], BF16, tag="v")
                o_sb = opool.tile([128, PPP * 3], F32, tag="o")
                nc.vector.tensor_tensor(
                    out=v_sb[:].rearrange("p (n c) -> p n c", c=3),
                    in0=y_sb[:].rearrange("p (n c) -> p n c", c=3),
                    in1=bco[:].unsqueeze(2).broadcast_to([128, PPP, 3]),
                    op=mybir.AluOpType.mult)
                nc.vector.tensor_tensor(out=o_sb[:], in0=v_sb[:], in1=u_sb[:],
                                        op=mybir.AluOpType.add)

                # ---- store f32 via HWDGE (no SWDGE descriptor cost) ----
                nc.sync.dma_start(o_v[b], o_sb[:])

    nc.finalize()
    return nc


_PROGRAM_CACHE = {}


def _get_program(npc, b01_nonzero, b2_nonzero, repeat=1):
    key = (npc, b01_nonzero, b2_nonzero, repeat)
    if key not in _PROGRAM_CACHE:
        _PROGRAM_CACHE[key] = build_program(npc, b01_nonzero, b2_nonzero,
                                            repeat=repeat)
    return _PROGRAM_CACHE[key]


def _shard_inputs(x, mask):
    """Split flat [NPX,...] arrays into per-core slices with the
    partition-contiguous pixel map applied implicitly (the DRAM views in
    build_program handle the in-core mapping; cores still take contiguous
    slabs)."""
    x_flat = np.ascontiguousarray(np.asarray(x, np.float32).reshape(NPX, IN_DIM))
    m_flat = np.ascontiguousarray(np.asarray(mask).astype(np.uint8).reshape(NPX))
    return x_flat, m_flat


def kernel(x, mask, w0, b0, w1, b1, w2, b2):
    consts = _prep_weights(
        np.asarray(w0, np.float32), np.asarray(b0, np.float32),
        np.asarray(w1, np.float32), np.asarray(b1, np.float32),
        np.asarray(w2, np.float32), np.asarray(b2, np.float32))
    x_flat, m_flat = _shard_inputs(x, mask)

    nc = _get_program(NPC, consts["b01_nonzero"], consts["b2_nonzero"])
    const_map = {k: np.asarray(v) for k, v in consts.items()
                 if k not in ("b2_nonzero", "b01_nonzero")}
    in_maps = []
    for k in range(N_CORES):
        lo, hi = k * NPC, (k + 1) * NPC
        in_maps.append({
            "x": x_flat[lo:hi],
            "mask": m_flat[lo:hi],
            **const_map,
        })
    res = run_bass_kernel_spmd(nc, in_maps, core_ids=list(range(N_CORES)))
    out = np.concatenate([res.results[k]["out"] for k in range(N_CORES)], axis=0)
    return out.reshape(B, H, W, OUT_DIM)


def _build_in_maps(x, mask, w0, b0, w1, b1, w2, b2, repeat=1):
    consts = _prep_weights(
        np.asarray(w0, np.float32), np.asarray(b0, np.float32),
        np.asarray(w1, np.float32), np.asarray(b1, np.float32),
        np.asarray(w2, np.float32), np.asarray(b2, np.float32))
    x_flat, m_flat = _shard_inputs(x, mask)
    nc = _get_program(NPC, consts["b01_nonzero"], consts["b2_nonzero"],
                      repeat=repeat)
    const_map = {k: np.asarray(v) for k, v in consts.items()
                 if k not in ("b2_nonzero", "b01_nonzero")}
    in_maps = []
    for k in range(N_CORES):
        lo, hi = k * NPC, (k + 1) * NPC
        in_maps.append({"x": x_flat[lo:hi], "mask": m_flat[lo:hi], **const_map})
    return nc, in_maps


if __name__ == "__main__":
    rng = np.random.default_rng(0)
    x = rng.random((B, H, W, IN_DIM), np.float32)
    mask = rng.integers(0, 2, (B, H, W)).astype(bool)
    w0 = rng.standard_normal((IN_DIM, HID)).astype(np.float32) * 0.5
    b0 = np.zeros(HID, np.float32)
    w1 = rng.standard_normal((HID, HID)).astype(np.float32) * 0.3
    b1 = np.zeros(HID, np.float32)
    w2 = rng.standard_normal((HID, OUT_DIM)).astype(np.float32) * 0.3
    b2 = np.zeros(OUT_DIM, np.float32)
    out = kernel(x=x, mask=mask, w0=w0, b0=b0, w1=w1, b1=b1, w2=w2, b2=b2)
    print("out", out.shape, out.dtype, out[0, 0, :2])


# revision 25
# speedup vs baseline: 18.4066x; 1.0218x over previous
"""ColorMLP Trainium2 kernel (v2 — engine-rebalanced).

Reference computation (per pixel, 8 input channels):
    h1 = relu(x @ w0 + b0)         # 8 -> 16
    h2 = relu(h1 @ w1 + b1)        # 16 -> 16
    y  = sigmoid(h2 @ w2 + b2)     # 16 -> 3
    out = mask * ((1-res)*rgb + res*y)   rgb = x[..,:3], res = x[..,3]

Strategy (pure data parallel over 8 cores, 1,048,576 px each):
  Pixel->partition map is partition-contiguous: partition p owns pixels
  [p*8192, (p+1)*8192) of the core's slice; batch b covers within-partition
  offsets [b*256, (b+1)*256).

  - mask: ONE SWDGE cast-DMA u8->bf16 for the whole core [128, 8192].
  - x: per-batch SWDGE cast-DMA f32->bf16 pixel-major [128, 2048].
  - PE transposes x chunks -> t_ps (bf16 PSUM); single DVE evac (2x mode).
  - L0/L1 matmuls write BF16 PSUM so evacs run in DVE 2x_1p / cheap ACT.
    relu1 evac on DVE (tensor_scalar max, 2x); relu2 evac on ACT (Relu).
  - L2 fused with output transpose (stationary = h2 chunk): z lands
    pixel-major f32 in PSUM; ACT sigmoid -> y_sb bf16.
  - blend split: nres+v+o on DVE, aco/bco/u on GpSimd (SBUF-only TTs).
  - store: o_sb f32 via HWDGE (nc.sync) — no SWDGE descriptor cost.
"""

import os
import sys

import numpy as np

sys.path.insert(0, "/opt/trn_rl_repo")

import ml_dtypes

import concourse.bacc as bacc
import concourse.bass as bass
import concourse.mybir as mybir
import concourse.tile as tile
from concourse.bass_utils import run_bass_kernel_spmd

F32 = mybir.dt.float32
BF16 = mybir.dt.bfloat16
U8 = mybir.dt.uint8

N_CORES = 8
B, H, W = 8, 1024, 1024
IN_DIM, HID, OUT_DIM = 8, 16, 3
NPX = B * H * W                  # 8388608
NPC = NPX // N_CORES             # 1048576 per core
PPPC = NPC // 128                # 8192 pixels per partition per core

BATCH_PX = 32768                 # pixels per batch
PPP = BATCH_PX // 128            # 256 px per partition per batch


def _bd(w, reps):
    """Block-diagonal of `w` repeated `reps` times: [reps*K, reps*M]."""
    k, m = w.shape
    out = np.zeros((reps * k, reps * m), np.float32)
    for g in range(reps):
        out[g * k:(g + 1) * k, g * m:(g + 1) * m] = w
    return out


def _prep_weights(w0, b0, w1, b1, w2, b2):
    """Host-side constant prep. Returns dict of named numpy arrays."""
    bf = ml_dtypes.bfloat16
    # W0T [128,128]: rows 0-63 & 64-127 are each blockdiag8(w0) [64,128]
    bd0 = _bd(w0, 8)  # [64, 128]
    w0t = np.concatenate([bd0, bd0], axis=0)  # [128, 128]
    w1bd = _bd(w1, 8)  # [128, 128]
    g2 = _bd(w2, 8)    # [128, 24]
    ident = np.eye(128, dtype=np.float32)
    b0col = np.tile(b0, 8).astype(np.float32).reshape(128, 1)
    b1col = np.tile(b1, 8).astype(np.float32).reshape(128, 1)
    # b2 pattern along the z free dim: slot = n*3 + ch, n in [0,256)
    b2row = np.tile(b2, 256).astype(np.float32).reshape(1, 768)
    return {
        "W0T": w0t.astype(bf),
        "W1BD": w1bd.astype(bf),
        "G2": g2.astype(bf),
        "IDENT": ident.astype(bf),
        "B0COL": b0col,
        "B1COL": b1col,
        "B2ROW": b2row,
        "b01_nonzero": bool(np.any(b0 != 0.0) or np.any(b1 != 0.0)),
        "b2_nonzero": bool(np.any(b2 != 0.0)),
    }


def build_program(npc, b01_nonzero, b2_nonzero, repeat=1):
    """Build the SPMD Bass program for one core processing `npc` pixels.

    `repeat` re-runs the whole pipeline R times (same input, same output)
    — bench-only knob to cancel host dispatch overhead out of wall-clock
    timing: device_time = (wall(R) - wall(1)) / (R - 1).
    """
    nc = bacc.Bacc("TRN2", target_bir_lowering=False, debug=False,
                   num_devices=N_CORES)
    n_batch = npc // BATCH_PX
    pppc = npc // 128

    x_d = nc.dram_tensor("x", [npc, IN_DIM], F32, kind="ExternalInput")
    m_d = nc.dram_tensor("mask", [npc], U8, kind="ExternalInput")
    w0t_d = nc.dram_tensor("W0T", [128, 128], BF16, kind="ExternalInput")
    w1bd_d = nc.dram_tensor("W1BD", [128, 128], BF16, kind="ExternalInput")
    g2_d = nc.dram_tensor("G2", [128, 24], BF16, kind="ExternalInput")
    id_d = nc.dram_tensor("IDENT", [128, 128], BF16, kind="ExternalInput")
    b0_d = nc.dram_tensor("B0COL", [128, 1], F32, kind="ExternalInput")
    b1_d = nc.dram_tensor("B1COL", [128, 1], F32, kind="ExternalInput")
    b2_d = nc.dram_tensor("B2ROW", [1, 768], F32, kind="ExternalInput")
    out_d = nc.dram_tensor("out", [npc, OUT_DIM], F32, kind="ExternalOutput")

    # DRAM views — partition-contiguous pixel map:
    # partition p owns pixels p*pppc + [0, pppc); batch b covers
    # within-partition offsets b*PPP + [0, PPP).
    x_v = x_d[:].rearrange("(p b n) f -> b p (n f)", p=128, b=n_batch)
    m_v = m_d[:].rearrange("(p n) -> p n", p=128)
    o_v = out_d[:].rearrange("(p b n) c -> b p (n c)", p=128, b=n_batch)

    with tile.TileContext(nc) as tc:
        with (
            tc.tile_pool(name="consts", bufs=1) as cpool,
            tc.tile_pool(name="xin", bufs=4) as xpool,
            tc.tile_pool(name="tsb", bufs=3) as tpool,
            tc.tile_pool(name="hsb", bufs=2) as hpool,
            tc.tile_pool(name="h2sb", bufs=2) as h2pool,
            tc.tile_pool(name="ysb", bufs=3) as ypool,
            tc.tile_pool(name="blend", bufs=3) as bpool,
            tc.tile_pool(name="osb", bufs=4) as opool,
            tc.tile_pool(name="tps", bufs=2, space="PSUM") as tps_pool,
            tc.tile_pool(name="hps", bufs=1, space="PSUM") as hps_pool,
            tc.tile_pool(name="h2ps", bufs=1, space="PSUM") as h2ps_pool,
            tc.tile_pool(name="zps", bufs=1, space="PSUM") as zps_pool,
        ):
            # ---- constants + whole-core mask (u8 -> bf16 cast DMA) ----
            w0t = cpool.tile([128, 128], BF16, tag="w0t")
            w1bd = cpool.tile([128, 128], BF16, tag="w1bd")
            g2 = cpool.tile([128, 24], BF16, tag="g2")
            ident = cpool.tile([128, 128], BF16, tag="ident")
            mask_sb = cpool.tile([128, pppc], BF16, tag="mask")
            nc.sync.dma_start(w0t[:], w0t_d[:])
            nc.sync.dma_start(w1bd[:], w1bd_d[:])
            nc.sync.dma_start(g2[:], g2_d[:])
            nc.sync.dma_start(ident[:], id_d[:])
            nc.gpsimd.dma_start(mask_sb[:], m_v)
            if b01_nonzero:
                b0c = cpool.tile([128, 1], F32, tag="b0c")
                b1c = cpool.tile([128, 1], F32, tag="b1c")
                nc.sync.dma_start(b0c[:], b0_d[:])
                nc.sync.dma_start(b1c[:], b1_d[:])
            if b2_nonzero:
                b2r = cpool.tile([1, 768], F32, tag="b2r")
                nc.sync.dma_start(b2r[:], b2_d[:])

            for b in [b for _ in range(repeat) for b in range(n_batch)]:
                # ---- load x (SWDGE cast f32->bf16), pixel-major ----
                x_bf = xpool.tile([128, PPP * IN_DIM], BF16, tag="x")  # [128,2048]
                nc.gpsimd.dma_start(x_bf[:], x_v[b])

                # ---- transposes: 16 chunks of [128,128], 2 waves -> t_sb ----
                t_sb = tpool.tile([128, 2048], BF16, tag="t")
                for w in range(2):
                    t_ps = tps_pool.tile([128, 1024], BF16, tag="tp")
                    for c in range(8):
                        nc.tensor.transpose(
                            t_ps[:, c * 128:(c + 1) * 128],
                            x_bf[:, (w * 8 + c) * 128:(w * 8 + c + 1) * 128],
                            ident[:],
                        )
                    nc.vector.tensor_copy(
                        t_sb[:, w * 1024:(w + 1) * 1024], t_ps[:])  # 2x

                # blend coefficients early — they only need x and mask, and
                # keeping them off the batch tail lets batches overlap.
                x3 = x_bf[:].rearrange("p (n f) -> p n f", f=IN_DIM)
                rgb = x3[:, :, 0:3]
                res = x3[:, :, 3]
                mk = mask_sb[:, b * PPP:(b + 1) * PPP]
                nres = bpool.tile([128, PPP], BF16, tag="nr")
                bco = bpool.tile([128, PPP], BF16, tag="bc")
                aco = bpool.tile([128, PPP], BF16, tag="ac")
                u_sb = bpool.tile([128, PPP * 3], BF16, tag="u")
                nc.vector.tensor_scalar(
                    out=nres[:], in0=res, scalar1=-1.0, scalar2=1.0,
                    op0=mybir.AluOpType.mult, op1=mybir.AluOpType.add)
                nc.gpsimd.tensor_tensor(out=bco[:], in0=res, in1=mk,
                                        op=mybir.AluOpType.mult)
                nc.gpsimd.tensor_tensor(out=aco[:], in0=nres[:], in1=mk,
                                        op=mybir.AluOpType.mult)
                u3 = u_sb[:].rearrange("p (n c) -> p n c", c=3)
                nc.gpsimd.tensor_tensor(
                    out=u3, in0=rgb,
                    in1=aco[:].unsqueeze(2).broadcast_to([128, PPP, 3]),
                    op=mybir.AluOpType.mult)

                y_sb = ypool.tile([128, PPP * 3], BF16, tag="y")       # [128,768]
                z_ps = zps_pool.tile([128, 768], F32, tag="z")

                def relu_evac(dst, src, bias_tile, on_act):
                    if on_act:
                        bias = bias_tile[:] if bias_tile is not None else 0.0
                        nc.scalar.activation(
                            dst, src, mybir.ActivationFunctionType.Relu,
                            bias=bias)
                    else:
                        s1 = bias_tile[:] if bias_tile is not None else 0.0
                        nc.vector.tensor_scalar(
                            out=dst, in0=src, scalar1=s1, scalar2=0.0,
                            op0=mybir.AluOpType.add, op1=mybir.AluOpType.max)

                # evac engine split across the 8 quarters (per batch):
                # 5 on ACT, 3 on DVE — roughly balances ACT (sigmoid) vs
                # DVE (t-evac + blend) load.
                r1_on_act = {0: True, 1: False, 2: True, 3: False}
                r2_on_act = {0: True, 1: True, 2: True, 3: False}

                for s in range(2):
                    b0t = b0c if b01_nonzero else None
                    b1t = b1c if b01_nonzero else None
                    h_sb = hpool.tile([128, 2048], BF16, tag=f"hs{s}")
                    for hh in range(2):
                        # ---- L0 (64-row tiles, f32 PSUM, N=512 x2) ----
                        h_ps = hps_pool.tile([128, 1024], F32, tag="h")
                        for q in range(2):
                            lo = hh * 1024 + q * 512
                            nc.tensor.matmul(
                                h_ps[:, q * 512:(q + 1) * 512],
                                lhsT=w0t[s * 64:(s + 1) * 64, :],
                                rhs=t_sb[s * 64:(s + 1) * 64, lo:lo + 512],
                            )
                        relu_evac(h_sb[:, hh * 1024:(hh + 1) * 1024], h_ps[:],
                                  b0t, r1_on_act[s * 2 + hh])

                    h2_sb = h2pool.tile([128, 2048], BF16, tag=f"h2s{s}")
                    for hh in range(2):
                        # ---- L1 (full array, f32 PSUM, N=512 x2) ----
                        h2_ps = h2ps_pool.tile([128, 1024], F32, tag="h2")
                        for q in range(2):
                            lo = hh * 1024 + q * 512
                            nc.tensor.matmul(
                                h2_ps[:, q * 512:(q + 1) * 512],
                                lhsT=w1bd[:],
                                rhs=h_sb[:, lo:lo + 512],
                            )
                        relu_evac(h2_sb[:, hh * 1024:(hh + 1) * 1024],
                                  h2_ps[:], b1t, r2_on_act[s * 2 + hh])

                    # ---- L2 fused with out-transpose ----
                    # chunk cc holds px n = cc*16 + s*8 + g (partition = j);
                    # z slot = n*3 + ch  ->  base col = cc*48 + s*24
                    for cc in range(16):
                        col = cc * 48 + s * 24
                        nc.tensor.matmul(
                            z_ps[:, col:col + 24],
                            lhsT=h2_sb[:, cc * 128:(cc + 1) * 128],
                            rhs=g2[:],
                        )

                if b2_nonzero:
                    nc.vector.tensor_tensor(
                        out=z_ps[:], in0=z_ps[:],
                        in1=b2r[:].partition_broadcast(128),
                        op=mybir.AluOpType.add)

                # ---- sigmoid + blend tail ----
                # 1-col touch absorbs the WAR wait on the recycled y_sb slot.
                nc.scalar.activation(
                    y_sb[:, 0:1], y_sb[:, 0:1],
                    mybir.ActivationFunctionType.Relu)
                v_sb = bpool.tile([128, PPP * 3], BF16, tag="v")
                o_sb = opool.tile([128, PPP * 3], F32, tag="o")
                # halve the tail: v/o of half 0 overlap sigmoid of half 1
                for zi in range(2):
                    sl = slice(zi * 384, (zi + 1) * 384)
                    ps = slice(zi * 128, (zi + 1) * 128)
                    nc.scalar.activation(
                        y_sb[:, sl], z_ps[:, sl],
                        mybir.ActivationFunctionType.Sigmoid,
                    )
                    nc.vector.tensor_tensor(
                        out=v_sb[:, sl].rearrange("p (n c) -> p n c", c=3),
                        in0=y_sb[:, sl].rearrange("p (n c) -> p n c", c=3),
                        in1=bco[:, ps].unsqueeze(2).broadcast_to(
                            [128, 128, 3]),
                        op=mybir.AluOpType.mult)
                    nc.vector.tensor_tensor(
                        out=o_sb[:, sl], in0=v_sb[:, sl], in1=u_sb[:, sl],
                        op=mybir.AluOpType.add)

                # ---- store f32 via HWDGE (no SWDGE descriptor cost) ----
                nc.sync.dma_start(o_v[b], o_sb[:])

    nc.finalize()
    return nc


_PROGRAM_CACHE = {}


def _get_program(npc, b01_nonzero, b2_nonzero, repeat=1):
    key = (npc, b01_nonzero, b2_nonzero, repeat)
    if key not in _PROGRAM_CACHE:
        _PROGRAM_CACHE[key] = build_program(npc, b01_nonzero, b2_nonzero,
                                            repeat=repeat)
    return _PROGRAM_CACHE[key]


def _shard_inputs(x, mask):
    """Split flat [NPX,...] arrays into per-core slices with the
    partition-contiguous pixel map applied implicitly (the DRAM views in
    build_program handle the in-core mapping; cores still take contiguous
    slabs)."""
    x_flat = np.ascontiguousarray(np.asarray(x, np.float32).reshape(NPX, IN_DIM))
    m_flat = np.ascontiguousarray(np.asarray(mask).astype(np.uint8).reshape(NPX))
    return x_flat, m_flat


def kernel(x, mask, w0, b0, w1, b1, w2, b2):
    consts = _prep_weights(
        np.asarray(w0, np.float32), np.asarray(b0, np.float32),
        np.asarray(w1, np.float32), np.asarray(b1, np.float32),
        np.asarray(w2, np.float32), np.asarray(b2, np.float32))
    x_flat, m_flat = _shard_inputs(x, mask)

    nc = _get_program(NPC, consts["b01_nonzero"], consts["b2_nonzero"])
    const_map = {k: np.asarray(v) for k, v in consts.items()
                 if k not in ("b2_nonzero", "b01_nonzero")}
    in_maps = []
    for k in range(N_CORES):
        lo, hi = k * NPC, (k + 1) * NPC
        in_maps.append({
            "x": x_flat[lo:hi],
            "mask": m_flat[lo:hi],
            **const_map,
        })
    res = run_bass_kernel_spmd(nc, in_maps, core_ids=list(range(N_CORES)))
    out = np.concatenate([res.results[k]["out"] for k in range(N_CORES)], axis=0)
    return out.reshape(B, H, W, OUT_DIM)


def _build_in_maps(x, mask, w0, b0, w1, b1, w2, b2, repeat=1):
    consts = _prep_weights(
        np.asarray(w0, np.float32), np.asarray(b0, np.float32),
        np.asarray(w1, np.float32), np.asarray(b1, np.float32),
        np.asarray(w2, np.float32), np.asarray(b2, np.float32))
    x_flat, m_flat = _shard_inputs(x, mask)
    nc = _get_program(NPC, consts["b01_nonzero"], consts["b2_nonzero"],
                      repeat=repeat)
    const_map = {k: np.asarray(v) for k, v in consts.items()
                 if k not in ("b2_nonzero", "b01_nonzero")}
    in_maps = []
    for k in range(N_CORES):
        lo, hi = k * NPC, (k + 1) * NPC
        in_maps.append({"x": x_flat[lo:hi], "mask": m_flat[lo:hi], **const_map})
    return nc, in_maps


if __name__ == "__main__":
    rng = np.random.default_rng(0)
    x = rng.random((B, H, W, IN_DIM), np.float32)
    mask = rng.integers(0, 2, (B, H, W)).astype(bool)
    w0 = rng.standard_normal((IN_DIM, HID)).astype(np.float32) * 0.5
    b0 = np.zeros(HID, np.float32)
    w1 = rng.standard_normal((HID, HID)).astype(np.float32) * 0.3
    b1 = np.zeros(HID, np.float32)
    w2 = rng.standard_normal((HID, OUT_DIM)).astype(np.float32) * 0.3
    b2 = np.zeros(OUT_DIM, np.float32)
    out = kernel(x=x, mask=mask, w0=w0, b0=b0, w1=w1, b1=b1, w2=w2, b2=b2)
    print("out", out.shape, out.dtype, out[0, 0, :2])
